# revision 11
# baseline (speedup 1.0000x reference)
"""Trainium2 Bass kernel for AnisotropicGaussianSampler.

Reference computation (H=W=128, N=4096 samples, B=8):
    corr[b,n] = (1/(H*W)) * sum_{h,w} A[b,h,w] * exp(-eh[h,n]) * exp(-ew[w,n])
    eh[h,n] = (h/H - mu[n,0])^2 / (2*sigma[n,0]^2)   (separable in h and w)

Factorization used on-device (per sample column n):
    Ph[h,n] = exp(-0.5 * zh^2),  zh = (mu0[n] - h/H) / sigma0[n]
    Pw[w,n] = exp(-0.5 * zw^2)
    N_b[w,n] = sum_h A[b,h,w] * Ph[h,n]          (matmul, lhsT = A_b as stored)
    corr[b,n] = (1/(H*W)) * sum_w Pw[w,n]*N_b[w,n]  (mul + ones-reduce matmul)

z is produced directly in PSUM by two accumulating K=1 matmuls with constant
lhsT rows {ones, -grid}: z[h,n] = 1*(mu/sigma)[n] + (-h/H)*(1/sigma)[n].
All compute-engine APs are partition-0 based (BIR verifier requirement).
Tensors feeding matmuls are typed float32r (single-pass fp32 matmul mode);
the producing instructions round to f32r as the verifier requires.

The final reduce accumulates all 8 batches into one [8,512] PSUM tile using
per-batch one-hot lhsT columns (start=b==0), so a single copy+DMA drains it.

Sharding: the 4096 sample points are split 512-per-core across 8 NeuronCores
(data-parallel in n); every core gets the full activations. Host concatenates
the per-core [8,512] outputs. No collectives needed.
"""

import os
import sys

import numpy as np

if "/opt/trn_rl_repo" not in sys.path:
    sys.path.insert(0, "/opt/trn_rl_repo")

B, H, W = 8, 128, 128
N_TOTAL = 4096
N_CORES = 8
NS = N_TOTAL // N_CORES  # 512 samples per core

# matmul input dtype mode: "f32r" (single-pass fp32), "f32" (4x slower), "bf16"
MM_MODE = os.environ.get("KERNEL_MM_MODE", "f32r")

LAST_EXEC_TIME_NS = None

_CACHE = {}


def _build_bass():
    import concourse.mybir as mybir
    import concourse.tile as tile
    from concourse import bacc

    f32 = mybir.dt.float32
    if MM_MODE == "f32r":
        mmdt = mybir.dt.float32r
    elif MM_MODE == "f32":
        mmdt = mybir.dt.float32
    else:
        mmdt = mybir.dt.bfloat16

    nc = bacc.Bacc()

    acts_d = nc.declare_dram_parameter("activations", [B, H, W], f32, isOutput=False)
    mu_d = nc.declare_dram_parameter("mu", [NS, 2], f32, isOutput=False)
    sig_d = nc.declare_dram_parameter("sigma", [NS, 2], f32, isOutput=False)
    # zconst row: [ones(H) | -grid(H)] on one partition
    zconst_d = nc.declare_dram_parameter("zconst", [1, 2 * H], f32, isOutput=False)
    oneh_d = nc.declare_dram_parameter("onehots", [W, B * B], f32, isOutput=False)
    out_d = nc.declare_dram_parameter("out", [B, NS], f32, isOutput=True)

    Exp = mybir.ActivationFunctionType.Exp
    Square = mybir.ActivationFunctionType.Square

    def cvt(pool, src, name):
        """Copy an f32 tile into an mmdt-typed tile (rounding for matmul)."""
        dst = pool.tile(list(src.shape), mmdt, tag=name)
        if len(src.shape) == 3:
            flat_src = src[:].rearrange("p a b -> p (a b)")
            flat_dst = dst[:].rearrange("p a b -> p (a b)")
        else:
            flat_src, flat_dst = src[:], dst[:]
        nc.vector.tensor_copy(flat_dst, flat_src)
        return dst

    with tile.TileContext(nc) as tc, nc.allow_low_precision(
        reason="float32r matmul inputs carry ~f32 precision"
    ):
        with (
            tc.tile_pool(name="const", bufs=1) as constp,
            tc.tile_pool(name="io", bufs=1) as iop,
            tc.tile_pool(name="sq", bufs=2) as sqp,
            tc.tile_pool(name="vbuf", bufs=3) as vp,
            tc.tile_pool(name="psz", bufs=2, space="PSUM") as pszp,
            tc.tile_pool(name="psn", bufs=2, space="PSUM") as psnp,
            tc.tile_pool(name="pso", bufs=1, space="PSUM") as psop,
        ):
            # ---- constant + input loads ----
            zconst_f = constp.tile([1, 2 * H], f32)
            nc.sync.dma_start(zconst_f[:], zconst_d[:])
            oneh_f = constp.tile([W, B * B], f32)
            nc.sync.dma_start(oneh_f[:], oneh_d[:])

            # mu/sigma as single-partition rows: [mu0(0..NS) | mu1(0..NS)]
            # interleaved layout [mu0[0], mu1[0], mu0[1], ...] — contiguous DMA
            murow = iop.tile([1, 2 * NS], f32)
            nc.sync.dma_start(
                murow[:], mu_d[:].rearrange("n t -> (n t)").unsqueeze(0)
            )
            sigrow = iop.tile([1, 2 * NS], f32)
            nc.sync.dma_start(
                sigrow[:], sig_d[:].rearrange("n t -> (n t)").unsqueeze(0)
            )

            acts_f = iop.tile([H, B, W], f32)
            for b in range(B):
                nc.sync.dma_start(acts_f[:, b, :], acts_d[b, :, :])

            # matmul-typed copies of constants and activations
            if MM_MODE == "f32":
                zconst, oneh, acts_mm = zconst_f, oneh_f, acts_f
            else:
                zconst = cvt(constp, zconst_f, "zc_mm")
                oneh = cvt(constp, oneh_f, "oneh_mm")
                acts_mm = cvt(iop, acts_f, "acts_mm")

            # ---- per-sample rows: recip = 1/sigma, m_is = mu/sigma ----
            reciprow = iop.tile([1, 2 * NS], mmdt)
            nc.vector.reciprocal(reciprow[:], sigrow[:])
            misrow = iop.tile([1, 2 * NS], mmdt)
            nc.vector.tensor_mul(misrow[:], murow[:], reciprow[:])

            # ---- z tables via two accumulating K=1 matmuls per axis ----
            tabdt = mmdt
            Ph = iop.tile([H, NS], tabdt)
            Pw = iop.tile([W, NS], tabdt)
            ones_c = zconst[:, 0:H]        # [1, H] constant ones
            negg_c = zconst[:, H:2 * H]    # [1, H] constant -h/H
            mis_tn = misrow[:].rearrange("one (n t) -> one n t", t=2)
            rec_tn = reciprow[:].rearrange("one (n t) -> one n t", t=2)
            for t, ptab in ((0, Ph), (1, Pw)):
                ps_z = pszp.tile([H, NS], f32, tag="z")
                nc.tensor.matmul(
                    ps_z[:], lhsT=ones_c, rhs=mis_tn[:, :, t],
                    start=True, stop=False,
                )
                nc.tensor.matmul(
                    ps_z[:], lhsT=negg_c, rhs=rec_tn[:, :, t],
                    start=False, stop=True,
                )
                sq = sqp.tile([H, NS], f32, tag="sq")
                nc.scalar.activation(sq[:], ps_z[:], Square)
                nc.scalar.activation(ptab[:], sq[:], Exp, scale=-0.5)

            # ---- per-batch: N_b = A_b^T-contract, V = Pw*N_b, reduce over w ----
            ps_out = psop.tile([B, NS], f32)
            for b in range(B):
                ps_n = psnp.tile([W, NS], f32, tag="n")
                nc.tensor.matmul(
                    ps_n[:], lhsT=acts_mm[:, b, :], rhs=Ph[:],
                    start=True, stop=True,
                )
                v = vp.tile([W, NS], tabdt, tag="v")
                nc.vector.tensor_mul(v[:], ps_n[:], Pw[:])
                nc.tensor.matmul(
                    ps_out[:], lhsT=oneh[:, b * B:(b + 1) * B], rhs=v[:],
                    start=(b == 0), stop=(b == B - 1),
                )

            # ---- scale by 1/(H*W), store ----
            out_sb = iop.tile([B, NS], f32)
            nc.scalar.mul(out_sb[:], ps_out[:], 1.0 / (H * W))
            nc.sync.dma_start(out_d[:], out_sb[:])

    nc.compile()
    return nc


def _constants():
    gh = np.arange(H, dtype=np.float32) / H
    zconst = np.concatenate([np.ones(H, np.float32), -gh]).reshape(1, 2 * H)
    zconst = np.ascontiguousarray(zconst.astype(np.float32))
    oneh = np.zeros((W, B * B), np.float32)
    for b in range(B):
        oneh[:, b * B + b] = 1.0
    return zconst, oneh


def kernel(activations, mu, sigma):
    from concourse.bass_utils import run_bass_kernel_spmd

    global LAST_EXEC_TIME_NS

    activations = np.ascontiguousarray(np.asarray(activations, dtype=np.float32))
    mu = np.ascontiguousarray(np.asarray(mu, dtype=np.float32))
    sigma = np.ascontiguousarray(np.asarray(sigma, dtype=np.float32))
    assert activations.shape == (B, H, W)
    assert mu.shape == (N_TOTAL, 2) and sigma.shape == (N_TOTAL, 2)

    if "nc" not in _CACHE:
        _CACHE["nc"] = _build_bass()
    nc = _CACHE["nc"]

    zconst, oneh = _constants()
    in_maps = []
    for c in range(N_CORES):
        sl = slice(c * NS, (c + 1) * NS)
        in_maps.append(
            {
                "activations": activations,
                "mu": np.ascontiguousarray(mu[sl]),
                "sigma": np.ascontiguousarray(sigma[sl]),
                "zconst": zconst,
                "onehots": oneh,
            }
        )

    res = run_bass_kernel_spmd(nc, in_maps, core_ids=list(range(N_CORES)))
    LAST_EXEC_TIME_NS = res.exec_time_ns

    out = np.concatenate([r["out"] for r in res.results], axis=1)  # [B, N_TOTAL]
    return out.reshape(B, 64, 64).astype(np.float32)


# revision 14
# speedup vs baseline: 1.0458x; 1.0458x over previous
"""Trainium2 Bass kernel for AnisotropicGaussianSampler.

Reference computation (H=W=128, N=4096 samples, B=8):
    corr[b,n] = (1/(H*W)) * sum_{h,w} A[b,h,w] * exp(-eh[h,n]) * exp(-ew[w,n])
    eh[h,n] = (h/H - mu[n,0])^2 / (2*sigma[n,0]^2)   (separable in h and w)

Factorization used on-device (per sample column n):
    Ph[h,n] = exp(-0.5 * zh^2),  zh = (mu0[n] - h/H) / sigma0[n]
    Pw[w,n] = exp(-0.5 * zw^2)
    N_b[w,n] = sum_h A[b,h,w] * Ph[h,n]          (matmul, lhsT = A_b as stored)
    corr[b,n] = (1/(H*W)) * sum_w Pw[w,n]*N_b[w,n]  (mul + ones-reduce matmul)

Table prep: 1/sigma and mu/sigma are computed across 128 partitions (fast DVE)
in a [128, 16] column tile, PE-transposed to [16, 128], copied to SBUF, and
DMA-gathered into four [1, 512] rows. Two accumulating K=1 matmuls per axis
then produce z directly in PSUM: z[h,n] = 1*(mu/sigma)[n] + (-h/H)*(1/sigma)[n],
and two ACT passes give exp(-0.5 z^2).

The batch loop is software-pipelined (skew 2) so the DVE multiply of batch b
overlaps the mm1 matmuls of batches b+1/b+2; the final reduce accumulates all
8 batches into one [8,512] PSUM tile via per-batch one-hot lhsT columns.

Sharding: the 4096 sample points are split 512-per-core across 8 NeuronCores
(data-parallel in n); every core gets the full activations. Host concatenates
the per-core [8,512] outputs. No collectives needed.
"""

import os
import sys

import numpy as np

if "/opt/trn_rl_repo" not in sys.path:
    sys.path.insert(0, "/opt/trn_rl_repo")

B, H, W = 8, 128, 128
N_TOTAL = 4096
N_CORES = 8
NS = N_TOTAL // N_CORES  # 512 samples per core
NCH = NS // 128          # n-chunks per core (4)

# matmul input dtype mode: "f32r" (single-pass fp32) or "f32" (4x slower)
MM_MODE = os.environ.get("KERNEL_MM_MODE", "f32r")

LAST_EXEC_TIME_NS = None

_CACHE = {}


def _build_bass():
    import concourse.mybir as mybir
    import concourse.tile as tile
    from concourse import bacc

    f32 = mybir.dt.float32
    mmdt = mybir.dt.float32r if MM_MODE == "f32r" else f32

    nc = bacc.Bacc()

    acts_d = nc.declare_dram_parameter("activations", [B, H, W], mmdt, isOutput=False)
    mu_d = nc.declare_dram_parameter("mu", [NS, 2], f32, isOutput=False)
    sig_d = nc.declare_dram_parameter("sigma", [NS, 2], f32, isOutput=False)
    # zconst row: [ones(H) | -grid(H)] on one partition
    zconst_d = nc.declare_dram_parameter("zconst", [1, 2 * H], mmdt, isOutput=False)
    oneh_d = nc.declare_dram_parameter("onehots", [W, B * B], mmdt, isOutput=False)
    ident_d = nc.declare_dram_parameter("ident", [128, 128], f32, isOutput=False)
    out_d = nc.declare_dram_parameter("out", [B, NS], f32, isOutput=True)

    Exp = mybir.ActivationFunctionType.Exp
    Square = mybir.ActivationFunctionType.Square

    with tile.TileContext(nc) as tc, nc.allow_low_precision(
        reason="float32r matmul inputs carry ~f32 precision"
    ):
        with (
            tc.tile_pool(name="const", bufs=1) as constp,
            tc.tile_pool(name="io", bufs=1) as iop,
            tc.tile_pool(name="sq", bufs=2) as sqp,
            tc.tile_pool(name="vbuf", bufs=4) as vp,
            tc.tile_pool(name="psz", bufs=2, space="PSUM") as pszp,
            tc.tile_pool(name="pst", bufs=1, space="PSUM") as pstp,
            tc.tile_pool(name="psn", bufs=4, space="PSUM") as psnp,
            tc.tile_pool(name="pso", bufs=1, space="PSUM") as psop,
        ):
            # ---- constant + input loads ----
            zconst = constp.tile([1, 2 * H], mmdt)
            nc.sync.dma_start(zconst[:], zconst_d[:])
            oneh = constp.tile([W, B * B], mmdt)
            nc.sync.dma_start(oneh[:], oneh_d[:])
            ident = constp.tile([128, 128], f32)
            nc.sync.dma_start(ident[:], ident_d[:])

            # mu/sigma in column layout [128, (t, c)]: elem (p, t, c) = mu[c*128+p, t]
            mu_cols = iop.tile([128, 2, NCH], f32)
            nc.sync.dma_start(
                mu_cols[:], mu_d[:].rearrange("(c p) t -> p t c", p=128)
            )
            sig_cols = iop.tile([128, 2, NCH], f32)
            nc.sync.dma_start(
                sig_cols[:], sig_d[:].rearrange("(c p) t -> p t c", p=128)
            )

            acts_sb = iop.tile([H, B, W], mmdt)
            for b in range(B):
                nc.sync.dma_start(acts_sb[:, b, :], acts_d[b, :, :])

            # ---- prep columns: [128, (q, t, c)] with q in {mu/sigma, 1/sigma} ----
            cols = iop.tile([128, 2, 2, NCH], f32)
            nc.vector.reciprocal(cols[:, 1, :, :], sig_cols[:])
            nc.vector.tensor_mul(cols[:, 0, :, :], mu_cols[:], cols[:, 1, :, :])

            # transpose all 16 columns at once -> [16, 128] rows
            tps = pstp.tile([2 * 2 * NCH, 128], f32)
            nc.tensor.transpose(
                tps[:], cols[:].rearrange("p q t c -> p (q t c)"), ident[:]
            )
            tsb = iop.tile([2 * 2 * NCH, 128], mmdt)
            nc.scalar.copy(tsb[:], tps[:])

            # gather rows: (q,t) -> [1, NS] with n-order (c, p)
            rows = {}
            for q in range(2):
                for t in range(2):
                    r = iop.tile([1, NS], mmdt, tag=f"row{q}{t}")
                    j = (q * 2 + t) * NCH
                    nc.sync.dma_start(
                        r[:].rearrange("one (c p) -> one c p", c=NCH),
                        tsb[j:j + NCH, :],
                    )
                    rows[(q, t)] = r

            # ---- z tables via two accumulating K=1 matmuls per axis ----
            Ph = iop.tile([H, NS], mmdt)
            Pw = iop.tile([W, NS], mmdt)
            ones_c = zconst[:, 0:H]        # [1, H] constant ones
            negg_c = zconst[:, H:2 * H]    # [1, H] constant -h/H
            for t, ptab in ((0, Ph), (1, Pw)):
                ps_z = pszp.tile([H, NS], f32, tag="z")
                nc.tensor.matmul(
                    ps_z[:], lhsT=ones_c, rhs=rows[(0, t)][:],
                    start=True, stop=False,
                )
                nc.tensor.matmul(
                    ps_z[:], lhsT=negg_c, rhs=rows[(1, t)][:],
                    start=False, stop=True,
                )
                sq = sqp.tile([H, NS], f32, tag="sq")
                nc.scalar.activation(sq[:], ps_z[:], Square)
                nc.scalar.activation(ptab[:], sq[:], Exp, scale=-0.5)

            # ---- pipelined batch loop (skew 2) ----
            SKEW = 2
            ps_out = psop.tile([B, NS], f32)
            ps_n = [None] * B

            def mm1(b):
                ps_n[b] = psnp.tile([W, NS], f32, tag="n", name=f"ps_n{b}")
                nc.tensor.matmul(
                    ps_n[b][:], lhsT=acts_sb[:, b, :], rhs=Ph[:],
                    start=True, stop=True,
                )

            for b in range(SKEW):
                mm1(b)
            for b in range(B):
                if b + SKEW < B:
                    mm1(b + SKEW)
                v = vp.tile([W, NS], mmdt, tag="v")
                nc.vector.tensor_mul(v[:], ps_n[b][:], Pw[:])
                nc.tensor.matmul(
                    ps_out[:], lhsT=oneh[:, b * B:(b + 1) * B], rhs=v[:],
                    start=(b == 0), stop=(b == B - 1),
                )

            # ---- scale by 1/(H*W), store ----
            out_sb = iop.tile([B, NS], f32)
            nc.scalar.mul(out_sb[:], ps_out[:], 1.0 / (H * W))
            nc.sync.dma_start(out_d[:], out_sb[:])

    nc.compile()
    return nc


def _constants():
    gh = np.arange(H, dtype=np.float32) / H
    zconst = np.concatenate([np.ones(H, np.float32), -gh]).reshape(1, 2 * H)
    zconst = np.ascontiguousarray(zconst.astype(np.float32))
    oneh = np.zeros((W, B * B), np.float32)
    for b in range(B):
        oneh[:, b * B + b] = 1.0
    ident = np.eye(128, dtype=np.float32)
    return zconst, oneh, ident


def kernel(activations, mu, sigma):
    from concourse.bass_utils import run_bass_kernel_spmd

    global LAST_EXEC_TIME_NS

    activations = np.ascontiguousarray(np.asarray(activations, dtype=np.float32))
    mu = np.ascontiguousarray(np.asarray(mu, dtype=np.float32))
    sigma = np.ascontiguousarray(np.asarray(sigma, dtype=np.float32))
    assert activations.shape == (B, H, W)
    assert mu.shape == (N_TOTAL, 2) and sigma.shape == (N_TOTAL, 2)

    if "nc" not in _CACHE:
        _CACHE["nc"] = _build_bass()
    nc = _CACHE["nc"]

    zconst, oneh, ident = _constants()
    in_maps = []
    for c in range(N_CORES):
        sl = slice(c * NS, (c + 1) * NS)
        in_maps.append(
            {
                "activations": activations,
                "mu": np.ascontiguousarray(mu[sl]),
                "sigma": np.ascontiguousarray(sigma[sl]),
                "zconst": zconst,
                "onehots": oneh,
                "ident": ident,
            }
        )

    res = run_bass_kernel_spmd(nc, in_maps, core_ids=list(range(N_CORES)))
    LAST_EXEC_TIME_NS = res.exec_time_ns

    out = np.concatenate([r["out"] for r in res.results], axis=1)  # [B, N_TOTAL]
    return out.reshape(B, 64, 64).astype(np.float32)
